# revision 1
# baseline (speedup 1.0000x reference)
"""Trainium2 Bass kernel for DeformableSincConv1d (v2, fp16 data path).

Data parallel over batch: 4 rows/core on 8 cores. Per core:
  1. Windowed im2col load (fp16): XX[l, j] = x_pad[10*l + j], j in [0,53)
  2. PE transposes (fp16 ident) -> psum [53, 1024] groups; evacuation copies
     apply the +1/+2 partition shifts: psum[1:52] -> X0P, psum[2:53] -> XPP.
     Batch-pair rows packed at partition bases 0 and 64 (legal SBUF starts).
  3. Offset conv: one matmul per 510-chunk, lhsT block-diag [115,115] fp16;
     Act evacuates psum with bias -> QS fp16; mask = is_ge(psum, -offb) on
     DVE straight from psum (no separate clip: max|offset| < 1, and the
     k=0 / k=50 boundary clips are realized exactly by zeroing Ep[50] and
     Em[0] rows).
  4. Sampling in polyphase layout end-to-end: X0P/XPP stored [115, 10, S]
     (plane r holds l = r mod 10); the transpose-evac copy performs the
     polyphase permutation for free via a rearranged dst AP. Per plane:
     q matmul + Act bias evac, mask from psum (DVE), Ep = XPP-X0P (Pool),
     Em = DMA partition-shift, copy_predicated (DVE), mul (Pool), per-row
     adds (Pool) -> dd planes, all with contiguous operands.
  5. Final conv: t0-outer per row, 51 rotated-filter matmuls with fp16
     CONTIGUOUS rhs dd[0:102, a, :ns]; dd[51:102] = plane-rotated DMA copy
     (stacked-102 rotated-filter decomposition); single strided psum->ysb
     copies alternating DVE/Act.
"""

import sys

import numpy as np

if "/opt/trn_rl_repo" not in sys.path:
    sys.path.insert(0, "/opt/trn_rl_repo")

SR = 16000
C_OUT = 80
K = 51
STRIDE = 10
HALF = (K - 1) // 2

B_FULL = 32
N_CORES = 8
B_LOC = B_FULL // N_CORES
L_FULL = 32000

R1 = 64          # partition base of second row in pair tiles
NP = R1 + K      # 115 rows in pair tiles


def _derive(L):
    L_out = (L - K) // STRIDE + 1
    T_out = (L_out * K - K) // STRIDE + 1
    NCHUNK = (L_out + 127) // 128
    LPAD = NCHUNK * 128
    XLEN = 10 * LPAD + 48
    return L_out, T_out, NCHUNK, LPAD, XLEN


def _host_filters(hz, band):
    hzc = np.clip(hz.astype(np.float32), 0.0, SR / 2).astype(np.float32)
    bandc = np.clip(band.astype(np.float32), 3.0, SR / 2).astype(np.float32)
    t_right = (np.arange(1, HALF + 1, dtype=np.float32) / np.float32(SR)).astype(np.float32)
    low = (hzc - bandc / 2).astype(np.float32)
    high = (hzc + bandc / 2).astype(np.float32)

    def sinc(t):
        ts = np.where(t == 0, np.float32(1.0), t)
        return np.where(t == 0, np.float32(1.0), np.sin(ts) / ts).astype(np.float32)

    a1 = (2 * high).astype(np.float32)
    a2 = (2 * low).astype(np.float32)
    bp_left = (a1 * sinc(a1 * t_right) - a2 * sinc(a2 * t_right)).astype(np.float32)
    bp = np.concatenate([bp_left, np.ones((C_OUT, 1), np.float32), bp_left[:, ::-1]], axis=1)
    return (bp / (2 * bandc)).astype(np.float32)  # [C_OUT, K]


def _host_f102(filt, L):
    """Stacked rotated filter matrices [128, K*C_OUT]; rows 0..50 = A-half
    (column offset a), rows 51..101 = B-half (column offset a+1, served by the
    column-shifted copy at dd[51:102])."""
    L_out, T_out, _, LPAD, _ = _derive(L)
    F = np.zeros((128, K, C_OUT), np.float32)
    for t0 in range(K):
        a = (STRIDE * t0) // K
        ns = (T_out - 1 - t0) // K + 1
        for k2 in range(K):
            kstar = (k2 + STRIDE * t0) % K
            lstar = (STRIDE * t0 + k2) // K
            if lstar == a:
                F[kstar, t0, :] = filt[:, k2]
            else:
                assert lstar == a + 1
                F[51 + kstar, t0, :] = filt[:, k2]
        assert a + 1 + STRIDE * (ns - 1) <= L_out - 1
        assert a + STRIDE * (ns - 1) <= STRIDE * ((T_out - 1) // K + 1) - 1
    return F.reshape(128, K * C_OUT)


def build_program(B_loc=B_LOC, L=L_FULL, debug=False):
    import concourse.bacc as bacc
    import concourse.tile as tile
    from concourse import bass, mybir

    f32 = mybir.dt.float32
    f16 = mybir.dt.float16
    u8 = mybir.dt.uint8
    Alu = mybir.AluOpType
    Act = mybir.ActivationFunctionType

    L_out, T_out, NCHUNK, LPAD, XLEN = _derive(L)
    NSMAX = (T_out - 1) // K + 1
    NPL = NSMAX                   # polyphase plane length (s slots)
    LSAMP = STRIDE * NPL          # sampled deformed region (covers all reads)
    assert LSAMP <= LPAD
    NG = (NCHUNK + 7) // 8          # transpose psum groups of 8 chunks
    CC = 510
    NCC = (LSAMP + CC - 1) // CC
    n_pairs = B_loc // 2

    nc = bacc.Bacc("TRN2", target_bir_lowering=False, debug=debug)

    x_d = nc.dram_tensor("x", [B_loc, XLEN], f16, kind="ExternalInput")
    wr2_d = nc.dram_tensor("wr2", [NP, NP], f16, kind="ExternalInput")
    offb2_d = nc.dram_tensor("offb2", [NP, 1], f32, kind="ExternalInput")
    negoffb2_d = nc.dram_tensor("negoffb2", [NP, 1], f32, kind="ExternalInput")
    f102_d = nc.dram_tensor("f102", [128, K * C_OUT], f16, kind="ExternalInput")
    ident_d = nc.dram_tensor("ident", [128, 128], f16, kind="ExternalInput")
    y_d = nc.dram_tensor("y", [B_loc, C_OUT, T_out], f32, kind="ExternalOutput")

    xap = x_d[:]

    with tile.TileContext(nc) as tc:
        with (
            tc.tile_pool(name="consts", bufs=1) as consts,
            tc.tile_pool(name="xxp", bufs=2) as xxp,
            tc.tile_pool(name="xkp", bufs=2) as xkp,
            tc.tile_pool(name="x0p", bufs=2) as x0p,
            tc.tile_pool(name="xpp", bufs=2) as xpp,
            tc.tile_pool(name="qsp", bufs=3) as qsp,
            tc.tile_pool(name="mtp", bufs=3) as mtp,
            tc.tile_pool(name="emp", bufs=7) as emp,
            tc.tile_pool(name="ddp", bufs=2) as ddp,
            tc.tile_pool(name="ysbp", bufs=2) as ysbp,
            tc.tile_pool(name="tpsum", bufs=2, space="PSUM") as tpsum,
            tc.tile_pool(name="qpsum", bufs=2, space="PSUM") as qpsum,
            tc.tile_pool(name="fpsum", bufs=4, space="PSUM") as fpsum,
        ):
            wr2_sb = consts.tile([NP, NP], f16)
            nc.sync.dma_start(out=wr2_sb[:], in_=wr2_d[:])
            offb2_sb = consts.tile([NP, 1], f32)
            nc.sync.dma_start(out=offb2_sb[:], in_=offb2_d[:])
            negoffb2_sb = consts.tile([NP, 1], f32)
            nc.sync.dma_start(out=negoffb2_sb[:], in_=negoffb2_d[:])
            f102_sb = consts.tile([128, K * C_OUT], f16)
            nc.sync.dma_start(out=f102_sb[:], in_=f102_d[:])
            ident_sb = consts.tile([128, 128], f16)
            nc.sync.dma_start(out=ident_sb[:], in_=ident_d[:])

            evac_engines = [nc.vector, nc.scalar]

            def ecopy(eng, dst, src):
                if eng is nc.scalar:
                    eng.copy(dst, src)
                else:
                    eng.tensor_copy(dst, src)

            def load_row(p, ri, X0P, XPP):
                r0 = 2 * p
                base = 0 if ri == 0 else R1
                xx = xxp.tile([128, NCHUNK, 53], f16, tag="xx")
                for g in range(NG):
                    h0 = 8 * g
                    hn = min(8, NCHUNK - h0)
                    in_ap = bass.AP(
                        tensor=xap.tensor,
                        offset=(r0 + ri) * XLEN + 1280 * h0,
                        ap=[[10, 128], [1280, hn], [1, 53]],
                    )
                    nc.sync.dma_start(out=xx[:, h0:h0 + hn, :], in_=in_ap)
                for g in range(NG):
                    n = min(8, NCHUNK - 8 * g)
                    pt = tpsum.tile([53, 1024], f16)
                    for c in range(n):
                        nc.tensor.transpose(pt[:, c * 128:(c + 1) * 128],
                                            xx[:, 8 * g + c, :], ident_sb[:])
                    xk = xkp.tile([53, 1024], f16)
                    ecopy(evac_engines[(p + ri + g) % 2], xk[:, :n * 128],
                          pt[:, :n * 128])
                    lo = g * 1024
                    nw = min(n * 128, LSAMP - lo)
                    if nw > 0:
                        nc.sync.dma_start(out=X0P[base:base + K, lo:lo + nw],
                                          in_=xk[1:52, :nw])
                        nc.sync.dma_start(out=XPP[base:base + K, lo:lo + nw],
                                          in_=xk[2:53, :nw])

            def finish_pair(X0P, XPP):
                gap = R1 - K
                nc.sync.dma_start(out=X0P[K:R1, :LSAMP], in_=X0P[0:gap, :LSAMP])
                nc.sync.dma_start(out=XPP[K:R1, :LSAMP], in_=XPP[0:gap, :LSAMP])
                # boundary clip realized as zeros: Ep[k=50] rows become 0
                nc.sync.dma_start(out=XPP[K - 1:K, :LSAMP], in_=X0P[K - 1:K, :LSAMP])
                nc.sync.dma_start(out=XPP[NP - 1:NP, :LSAMP], in_=X0P[NP - 1:NP, :LSAMP])
                # XPP becomes Ep in place; off the per-chunk critical path
                for c7 in range(NCC):
                    n = min(CC, LSAMP - c7 * CC)
                    sl = slice(c7 * CC, c7 * CC + n)
                    nc.gpsimd.tensor_sub(XPP[:, sl], XPP[:, sl], X0P[:, sl])

            def alloc_pair():
                X0P = x0p.tile([NP, LPAD], f16)
                XPP = xpp.tile([NP, LPAD], f16)
                dd0 = ddp.tile([102, STRIDE, NPL], f16, tag="dd")
                dd1 = ddp.tile([102, STRIDE, NPL], f16, tag="dd")
                return X0P, XPP, dd0, dd1

            def front_chunk(state, c7, vadd=False, do_adds=True):
                """Sampling chain for one 510-column l-chunk."""
                X0P, XPP, dd0, dd1 = state
                n = min(CC, LSAMP - c7 * CC)
                sl = slice(c7 * CC, c7 * CC + n)
                qp = qpsum.tile([NP, CC], f32)
                nc.tensor.matmul(qp[:, :n], wr2_sb[:], X0P[:, sl],
                                 start=True, stop=True)
                QSC = qsp.tile([NP, CC], f16)
                nc.scalar.activation(QSC[:, :n], qp[:, :n], Act.Identity,
                                     bias=offb2_sb[:])
                MTC = mtp.tile([NP, CC], u8)
                nc.vector.tensor_scalar(MTC[:, :n], qp[:, :n],
                                        negoffb2_sb[:], None, op0=Alu.is_ge)
                EP = XPP[:, sl]
                EMC = emp.tile([NP, CC], f16)
                nc.sync.dma_start(out=EMC[0:1, :n], in_=EP[K - 1:K])
                nc.sync.dma_start(out=EMC[1:K, :n], in_=EP[0:K - 1])
                nc.sync.dma_start(out=EMC[R1:R1 + 1, :n], in_=EP[NP - 1:NP])
                nc.sync.dma_start(out=EMC[R1 + 1:NP, :n], in_=EP[R1:NP - 1])
                nc.vector.copy_predicated(EMC[:, :n], MTC[:, :n], EP)
                nc.gpsimd.tensor_mul(EMC[:, :n], QSC[:, :n], EMC[:, :n])
                s0, nS = c7 * CC // STRIDE, n // STRIDE
                dv0 = dd0[0:K, :, s0:s0 + nS].rearrange("p r s -> p s r")
                dv1 = dd1[0:K, :, s0:s0 + nS].rearrange("p r s -> p s r")
                if not do_adds:
                    return EMC
                eng0 = nc.vector if vadd else nc.gpsimd
                eng0.tensor_add(dv0, X0P[0:K, sl], EMC[0:K, :n])
                nc.gpsimd.tensor_add(dv1, X0P[R1:NP, sl], EMC[R1:NP, :n])
                nc.sync.dma_start(out=dd0[K:102, 0:9, s0:s0 + nS],
                                  in_=dd0[0:K, 1:10, s0:s0 + nS])
                nc.sync.dma_start(out=dd1[K:102, 0:9, s0:s0 + nS],
                                  in_=dd1[0:K, 1:10, s0:s0 + nS])

            def chunk_adds(state, emcs, which):
                X0P, XPP, dd0, dd1 = state
                dd = dd0 if which == 0 else dd1
                base = 0 if which == 0 else R1
                for c7 in range(NCC):
                    n = min(CC, LSAMP - c7 * CC)
                    sl = slice(c7 * CC, c7 * CC + n)
                    s0, nS = c7 * CC // STRIDE, n // STRIDE
                    dv = dd[0:K, :, s0:s0 + nS].rearrange("p r s -> p s r")
                    nc.gpsimd.tensor_add(dv, X0P[base:base + K, sl],
                                         emcs[c7][base:base + K, :n])
                    nc.sync.dma_start(out=dd[K:102, 0:9, s0:s0 + nS],
                                      in_=dd[0:K, 1:10, s0:s0 + nS])
                nc.sync.dma_start(out=dd[K:102, 9, 0:NPL - 1],
                                  in_=dd[0:K, 0, 1:NPL])
                return dd

            def front_tail(state):
                """B-half plane rotation for both dd tiles."""
                _, _, dd0, dd1 = state
                for dd in (dd0, dd1):
                    nc.sync.dma_start(out=dd[K:102, 9, 0:NPL - 1],
                                      in_=dd[0:K, 0, 1:NPL])
                return dd0, dd1

            SCATTER = {0: nc.scalar, 1: nc.vector}

            def final_t0(r, dd, ysb, t0):
                a = (STRIDE * t0) // K
                ns = (T_out - 1 - t0) // K + 1
                fp = fpsum.tile([C_OUT, NSMAX], f32)
                rhs = dd[0:102, a, 0:ns]
                lhsT = f102_sb[0:102, t0 * C_OUT:(t0 + 1) * C_OUT]
                nc.tensor.matmul(fp[:, :ns], lhsT, rhs, start=True, stop=True)
                yv = ysb[:, t0:t0 + K * (ns - 1) + 1:K]
                ecopy(SCATTER[(r + t0) % 2], yv, fp[:, :ns])

            HH = NSMAX // 2   # s-half size (160)

            def final_t0pair(r, dd, ysb, t0):
                """t0 and t0+1 interleaved in one psum bank per s-half; the
                evacuation writes (s, t0)-pairs so consecutive stores are
                8-byte adjacent in ysb."""
                a0 = (STRIDE * t0) // K
                a1 = (STRIDE * (t0 + 1)) // K
                ns0 = (T_out - 1 - t0) // K + 1
                ns1 = (T_out - 1 - (t0 + 1)) // K + 1
                l0 = f102_sb[0:102, t0 * C_OUT:(t0 + 1) * C_OUT]
                l1 = f102_sb[0:102, (t0 + 1) * C_OUT:(t0 + 2) * C_OUT]
                for h in range(2):
                    s_lo = h * HH
                    n0 = min(ns0 - s_lo, HH)
                    n1 = min(ns1 - s_lo, HH)
                    fp = fpsum.tile([C_OUT, 2, HH], f32)
                    nc.tensor.matmul(fp[:, 0, :n0], l0,
                                     dd[0:102, a0, s_lo:s_lo + n0],
                                     start=True, stop=True)
                    nc.tensor.matmul(fp[:, 1, :n1], l1,
                                     dd[0:102, a1, s_lo:s_lo + n1],
                                     start=True, stop=True)
                    base = t0 + K * s_lo
                    yv2 = ysb[:, base:base + K * n1].rearrange(
                        "p (s q) -> p s q", q=K)[:, :, 0:2]
                    sv = fp[:, :, :n1].rearrange("p t s -> p s t")
                    ecopy(SCATTER[(r + t0 + h) % 2], yv2, sv)
                    if n0 > n1:
                        yt = ysb[:, base + K * n1:base + K * n1 + 1]
                        ecopy(SCATTER[(r + t0 + h + 1) % 2], yt,
                              fp[:, 0, n1:n0])

            def final_rowpass(r, dd, ysb):
                for t0 in range(0, K - 1, 2):
                    final_t0pair(r, dd, ysb, t0)
                final_t0(r, dd, ysb, K - 1)

            def final_pair_h(r, dd, ysb, t0, h):
                a0 = (STRIDE * t0) // K
                a1 = (STRIDE * (t0 + 1)) // K
                ns0 = (T_out - 1 - t0) // K + 1
                ns1 = (T_out - 1 - (t0 + 1)) // K + 1
                l0 = f102_sb[0:102, t0 * C_OUT:(t0 + 1) * C_OUT]
                l1 = f102_sb[0:102, (t0 + 1) * C_OUT:(t0 + 2) * C_OUT]
                s_lo = h * HH
                n0 = min(ns0 - s_lo, HH)
                n1 = min(ns1 - s_lo, HH)
                fp = fpsum.tile([C_OUT, 2, HH], f32)
                nc.tensor.matmul(fp[:, 0, :n0], l0,
                                 dd[0:102, a0, s_lo:s_lo + n0],
                                 start=True, stop=True)
                nc.tensor.matmul(fp[:, 1, :n1], l1,
                                 dd[0:102, a1, s_lo:s_lo + n1],
                                 start=True, stop=True)
                base = t0 + K * s_lo
                yv2 = ysb[:, base:base + K * n1].rearrange(
                    "p (s q) -> p s q", q=K)[:, :, 0:2]
                sv = fp[:, :, :n1].rearrange("p t s -> p s t")
                ecopy(SCATTER[(r + t0 + h) % 2], yv2, sv)
                if n0 > n1:
                    yt = ysb[:, base + K * n1:base + K * n1 + 1]
                    ecopy(SCATTER[(r + t0 + h + 1) % 2], yt, fp[:, 0, n1:n0])

            def final_last_h(r, dd, ysb, h):
                t0 = K - 1
                a = (STRIDE * t0) // K
                ns = (T_out - 1 - t0) // K + 1
                s_lo = h * HH
                n = min(ns - s_lo, HH)
                fp = fpsum.tile([C_OUT, 2, HH], f32)
                lhsT = f102_sb[0:102, t0 * C_OUT:(t0 + 1) * C_OUT]
                nc.tensor.matmul(fp[:, 0, :n], lhsT,
                                 dd[0:102, a, s_lo:s_lo + n],
                                 start=True, stop=True)
                yv = ysb[:, t0 + K * s_lo:t0 + K * (s_lo + n - 1) + 1:K]
                ecopy(SCATTER[(r + h) % 2], yv, fp[:, 0, :n])

            st0 = alloc_pair()
            load_row(0, 0, st0[0], st0[1])
            load_row(0, 1, st0[0], st0[1])
            finish_pair(st0[0], st0[1])
            st1 = alloc_pair()
            for c7 in range(NCC):
                front_chunk(st0, c7, vadd=True)
                if c7 == 0:
                    load_row(1, 0, st1[0], st1[1])
                elif c7 == 2:
                    load_row(1, 1, st1[0], st1[1])
            if NCC <= 2:
                load_row(1, 1, st1[0], st1[1])
            dd0, dd1 = front_tail(st0)
            finish_pair(st1[0], st1[1])

            # final rows 0,1 with pair-1 sampling (sans adds) interleaved
            YSBW = T_out + K - 1
            ysb0 = ysbp.tile([C_OUT, YSBW], f32, tag="ysb")
            ysb1 = ysbp.tile([C_OUT, YSBW], f32, tag="ysb")
            emcs1 = []
            for i, t0 in enumerate(range(0, K - 1, 2)):
                final_t0pair(0, dd0, ysb0, t0)
                if i < NCC:
                    emcs1.append(front_chunk(st1, i, do_adds=False))
            final_t0(0, dd0, ysb0, K - 1)
            nc.sync.dma_start(out=y_d[0], in_=ysb0[:, :T_out])
            # dd2 reuses dd0's buffer (frees after final(0)); adds run during
            # final(1) on the otherwise-idle Pool engine
            dd2 = chunk_adds(st1, emcs1, 0)
            dd3 = chunk_adds(st1, emcs1, 1)
            final_rowpass(1, dd1, ysb1)
            nc.sync.dma_start(out=y_d[1], in_=ysb1[:, :T_out])

            ysb2 = ysbp.tile([C_OUT, YSBW], f32, tag="ysb")
            final_rowpass(2, dd2, ysb2)
            nc.sync.dma_start(out=y_d[2], in_=ysb2[:, :T_out])
            ysb3 = ysbp.tile([C_OUT, YSBW], f32, tag="ysb")
            final_rowpass(3, dd3, ysb3)
            nc.sync.dma_start(out=y_d[3], in_=ysb3[:, :T_out])

    nc.compile()
    return nc


def _host_inputs(x, hz, band, offset_w, offset_b, B_loc, L):
    """Build the per-core input maps."""
    L_out, T_out, NCHUNK, LPAD, XLEN = _derive(L)
    filt = _host_filters(hz, band)
    f102 = _host_f102(filt, L).astype(np.float16)
    wr = offset_w[:, 0, :].T.astype(np.float32)  # [k_in, k_out]
    wr2 = np.zeros((NP, NP), np.float32)
    wr2[0:K, 0:K] = wr
    wr2[R1:NP, R1:NP] = wr
    offb2 = np.zeros((NP, 1), np.float32)
    offb2[0:K, 0] = offset_b.astype(np.float32)
    offb2[R1:NP, 0] = offset_b.astype(np.float32)
    negoffb2 = -offb2
    ident = np.eye(128, dtype=np.float16)

    B = x.shape[0]
    xpad = np.zeros((B, XLEN), np.float16)
    xpad[:, 1:1 + L] = x.astype(np.float16)

    n_cores = B // B_loc
    in_maps = []
    for i in range(n_cores):
        in_maps.append({
            "x": np.ascontiguousarray(xpad[i * B_loc:(i + 1) * B_loc]),
            "wr2": wr2.astype(np.float16),
            "offb2": offb2,
            "negoffb2": negoffb2,
            "f102": f102,
            "ident": ident,
        })
    return in_maps


_CACHED = {}


def _get_program():
    key = (B_LOC, L_FULL)
    if key not in _CACHED:
        _CACHED[key] = build_program(B_LOC, L_FULL)
    return _CACHED[key]


def kernel(x, hz, band, offset_w, offset_b):
    from concourse.bass_utils import run_bass_kernel_spmd

    x = np.asarray(x, dtype=np.float32)
    hz = np.asarray(hz, dtype=np.float32)
    band = np.asarray(band, dtype=np.float32)
    offset_w = np.asarray(offset_w, dtype=np.float32)
    offset_b = np.asarray(offset_b, dtype=np.float32)

    nc = _get_program()
    in_maps = _host_inputs(x, hz, band, offset_w, offset_b, B_LOC, L_FULL)
    res = run_bass_kernel_spmd(nc, in_maps, list(range(N_CORES)))
    outs = [res.results[i]["y"] for i in range(N_CORES)]
    return np.concatenate(outs, axis=0)

